# revision 17
# baseline (speedup 1.0000x reference)
"""Trainium2 Bass kernel for BbBartAttention (sparse relative-position bias).

Sharding: 8 cores = 4 batches x 2 head-groups (6 heads each).
Each core computes, for its (batch b, head-group g):
  q/k/v projections (transposed layouts, bf16), per-head biased attention
  scores, softmax (no max-subtraction; logits are O(1)), PV with a ones-column
  to get softmax denominators, normalization, and a partial output projection
  over its 384 head-dims. Host sums the two group partials per batch and
  adds the (o_b + v_b @ o_w_g.T) row, which is exact because softmax rows
  sum to 1.

Bias structure: with A[i,j] = [pos_row_i==pos_row_j], C[i,j] = [pos_col_i==
pos_col_j], D = A*C, the reference's table index is exactly A + 2*C, so
  bias = t0 + (t1-t0)*A + (t2-t0)*C + (t3-t1-t2+t0)*D
All three data-dependent terms are applied in ONE fp8 DoubleRow matmul per
score block (2x column throughput, 256-deep contraction): subtile 0 pairs a
scaled identity (lhsT) with the D matrix (rhs), subtile 1 pairs the
per-head-scaled one-hot stack [c1*R; c2*P] (lhsT) with [R; P] (rhs). All fp8
operands are exact ({0,1} one-hots, D) or tiny coefficients scaled x8 (the
exp activation's scale=1/8 undoes this while also applying the 1/sqrt(64)
q-scaling, so q/k stay unscaled). D = relu(gram([R;P]) - 1) is materialized
once per batch directly in fp8, with the gram itself a 2x64-subtile fp8
DoubleRow matmul.

Scores are computed transposed, S^T[j, i] (key index on partitions), so the
softmax denominator falls out of the PV matmul via a ones-column appended to
V, and the attention output lands directly in the [head_dim, i] layout the
output projection needs as lhsT. Output is produced transposed [E, S] in
bf16; the host transposes back. All activations/weights ride bf16 to halve
DMA and LDWEIGHTS time.

Scheduling notes (the PE executes its queue strictly in order; each
dma_start costs ~0.6us on the issuing queue; the PE clock ramps up only
under sustained load and falls back on multi-us idles):
 - Matmuls feed 1024 output columns wherever the operand layout allows,
   halving instruction count.
 - Each score unit's PV matmul is emitted AFTER the next unit's QK+bias
   matmuls (software pipelining), so the PE never head-of-line blocks on
   the exp it depends on.
 - Inputs are coalesced DMAs on the sync queue in consumption order.
 - Phase 3 runs inside the phase-2 pool scope, its accumulators riding the
   PV psum ring (no pool barrier); its xT0/xT1 matmuls plus a burst of
   no-op filler matmuls keep the PE busy (and clocked up) through the
   normalization bounce of the last head pair.
"""

import numpy as np
from contextlib import ExitStack

import ml_dtypes
import concourse.bass as bass
import concourse.tile as tile
from concourse import bacc, mybir
from concourse.bass_utils import run_bass_kernel_spmd

F32 = mybir.dt.float32
F32R = mybir.dt.float32r
BF16 = mybir.dt.bfloat16
F8 = mybir.dt.float8e4
AF = mybir.ActivationFunctionType
ALU = mybir.AluOpType
DR_MODE = mybir.MatmulPerfMode.DoubleRow

B, S, E, H = 4, 1024, 768, 12
D_HEAD = 64
SCALING = D_HEAD ** -0.5
HG = 2            # head groups (tensor-parallel)
HPG = H // HG     # 6 heads per group
GD = HPG * D_HEAD # 384 head-dims per group
NROW, NCOL = 64, 32
RP = NROW + NCOL  # 96
KT = E // 128     # 6 contraction tiles for projections
MT = GD // 128    # 3 m-tiles for Q^T/K^T
JT = S // 128     # 8 key tiles
IT = S // 512     # 2 query column tiles

_CACHE = {}


def build_nc():
    if "nc" in _CACHE:
        return _CACHE["nc"]
    nc = bacc.Bacc("TRN2", target_bir_lowering=False, debug=False, num_devices=8)

    def inp(name, shape, dt=F32):
        return nc.dram_tensor(name, shape, dt, kind="ExternalInput").ap()

    x_hsT = inp("hsT", [E, S], BF16)
    x_wqT = inp("wqT", [E, GD], BF16)
    x_wkT = inp("wkT", [E, GD], BF16)
    x_wvT = inp("wvT", [E, GD], BF16)
    x_woT = inp("woT", [GD, E], BF16)
    x_rptg = inp("rptg", [64, 2 * S], F8)        # [R | P-padded] gram pair
    x_rpt96 = inp("rpt96r", [128, JT * S], F8)   # rpt96 replicated 8x
    x_lt = inp("lt", [128, HPG * JT * 2 * 128], F8)
    x_meta = inp("meta", [128, 2 * MT + HPG])    # [qb | kb | c0]
    x_ones = inp("ones64", [1, 64], BF16)
    y_out = nc.dram_tensor("outp", [E, S], BF16, kind="ExternalOutput").ap()

    with tile.TileContext(nc) as tc:
        with ExitStack() as ctx:
            cp = ctx.enter_context(tc.tile_pool(name="const", bufs=1))

            # ---- persistent SBUF tensors ----
            ones64 = cp.tile([1, 64], BF16, tag="ones64")
            rptg = cp.tile([64, 2 * S], F8, tag="rptg")
            meta = cp.tile([128, 2 * MT + HPG], F32, tag="meta")
            qb = meta[:, 0:MT]
            kb = meta[:, MT:2 * MT]
            c0 = meta[:, 2 * MT:2 * MT + HPG]
            # per-(h,j): [ic3_h | rps_h-block] as a DoubleRow lhsT pair
            LT = cp.tile([128, HPG * JT * 2 * 128], F8, tag="LT")
            # per-j: [Dm_j | rpt96] as a DoubleRow rhs pair
            DRA = cp.tile([128, JT * 2 * S], F8, tag="DRA")
            hsA = cp.tile([128, KT * S], BF16, tag="hsA")
            wqA = cp.tile([128, KT * GD], BF16, tag="wqA")
            wkA = cp.tile([128, KT * GD], BF16, tag="wkA")
            wvA = cp.tile([128, KT * GD], BF16, tag="wvA")
            woA = cp.tile([128, MT * E], BF16, tag="woA")
            # upper half of woT[2] staged at partition base 0: phase 3 pairs
            # it with nm2 (also base 0) since fmap and weights must share a
            # partition base
            woT2b = cp.tile([64, E], BF16, tag="woT2b")
            QT = [cp.tile([128, S], BF16, tag=f"QT{m}", name=f"QT{m}") for m in range(MT)]
            KTt = [cp.tile([128, S], BF16, tag=f"KTt{m}", name=f"KTt{m}") for m in range(MT)]
            # V with interleaved ones column per head: [V_h (64) | 1], 6*65=390
            V = [cp.tile([128, HPG * 65], BF16, tag=f"V{j}", name=f"V{j}") for j in range(JT)]
            xT = [cp.tile([128, S], BF16, tag=f"xT{m}", name=f"xT{m}") for m in range(MT)]

            ltv = LT[:].rearrange("p (h j t c) -> p h j t c", h=HPG, j=JT, c=128)
            drv = DRA[:].rearrange("p (j t c) -> p j t c", j=JT, c=S)
            rgv = rptg[:].rearrange("p (t c) -> p t c", t=2)
            hsv = hsA[:].rearrange("p (k c) -> p k c", k=KT)
            wqv = wqA[:].rearrange("p (k c) -> p k c", k=KT)
            wkv = wkA[:].rearrange("p (k c) -> p k c", k=KT)
            wvv = wvA[:].rearrange("p (k c) -> p k c", k=KT)
            wov = woA[:].rearrange("p (m c) -> p m c", m=MT)

            def slabs(x, n, w, grp):
                # DRAM [n*128, w] -> [128, grp, w] views for coalesced DMAs
                out = []
                for k0 in range(0, n, grp):
                    g = min(grp, n - k0)
                    out.append(x[k0 * 128:(k0 + g) * 128, :].rearrange(
                        "(k p) c -> p k c", k=g))
                return out

            # ---- phase 1: projections + bias prep ----
            with ExitStack() as p1:
                # One sync-queue DMA stream in consumption order.
                nc.sync.dma_start(ones64[:], x_ones)
                nc.sync.dma_start(rptg[:], x_rptg)
                nc.sync.dma_start(meta[:], x_meta)
                for i, src in enumerate(slabs(x_hsT, KT, S, 2)):
                    nc.sync.dma_start(hsv[:, 2 * i:2 * i + 2, :], src)
                for i, src in enumerate(slabs(x_wqT, KT, GD, 3)):
                    nc.sync.dma_start(wqv[:, 3 * i:3 * i + 3, :], src)
                for i, src in enumerate(slabs(x_wkT, KT, GD, 3)):
                    nc.sync.dma_start(wkv[:, 3 * i:3 * i + 3, :], src)
                for i, src in enumerate(slabs(x_wvT, KT, GD, 3)):
                    nc.sync.dma_start(wvv[:, 3 * i:3 * i + 3, :], src)
                nc.sync.dma_start(drv[:, :, 1, :], x_rpt96[:].rearrange(
                    "p (j c) -> p j c", j=JT))
                nc.sync.dma_start(LT[:], x_lt)
                for src in slabs(x_woT, MT, E, 3):
                    nc.sync.dma_start(wov[:, :, :], src)
                nc.sync.dma_start(woT2b[:], x_woT[(MT - 1) * 128 + 64:MT * 128, :])

                ps = p1.enter_context(tc.tile_pool(name="ps1", bufs=2, space="PSUM"))

                # A few no-op matmuls as soon as ones64 lands: starts the
                # PE clock ramp while the real inputs are still in flight.
                warm = ps.tile([64, 64], F32, tag="warm", bufs=1)
                for _ in range(24):
                    nc.tensor.matmul(warm[:], ones64[:], ones64[:],
                                     start=True, stop=True)

                # D = relu((A+C) - 1), straight to fp8 in the DoubleRow rhs;
                # the gram is itself fp8 DoubleRow over 2x64-row subtiles.
                for j in range(JT):
                    acc = ps.tile([128, S], F32, tag="gram")
                    for i2 in range(IT):
                        nc.tensor.matmul(
                            acc[:, i2 * 512:(i2 + 1) * 512],
                            rgv[:, :, j * 128:(j + 1) * 128],
                            rgv[:, :, i2 * 512:(i2 + 1) * 512],
                            start=True, stop=True,
                            perf_mode=DR_MODE)
                    nc.vector.tensor_scalar(
                        drv[:, j, 0, :], acc[:],
                        -1.0, 0.0, ALU.add, ALU.max)
                for m in range(MT):
                    for (wv_, dst, bias) in [(wqv, QT, qb), (wkv, KTt, kb)]:
                        acc = ps.tile([128, S], F32, tag="gram")
                        for i2 in range(IT):
                            for k in range(KT):
                                nc.tensor.matmul(
                                    acc[:, i2 * 512:(i2 + 1) * 512],
                                    wv_[:, k, m * 128:(m + 1) * 128],
                                    hsv[:, k, i2 * 512:(i2 + 1) * 512],
                                    start=(k == 0), stop=(k == KT - 1))
                        nc.scalar.activation(
                            dst[m][:], acc[:],
                            AF.Identity, bias=bias[:, m:m + 1])
                for j in range(JT):
                    acc = ps.tile([128, GD], F32, tag="projv")
                    for k in range(KT):
                        nc.tensor.matmul(
                            acc[:],
                            hsv[:, k, j * 128:(j + 1) * 128],
                            wvv[:, k, :],
                            start=(k == 0), stop=(k == KT - 1))
                    # strided copy into [V_h | ones] layout
                    vv = V[j][:].rearrange("p (h c) -> p h c", c=65)
                    av = acc[:].rearrange("p (h c) -> p h c", c=64)
                    nc.vector.tensor_copy(vv[:, :, 0:64], av)
                    # ones column; walrus rejects MEMSET on some dtypes, so
                    # use (x * 0) + 1 via tensor_scalar instead
                    nc.vector.tensor_scalar(
                        vv[:, :, 64:65], av[:, :, 0:1], 0.0, 1.0,
                        ALU.mult, ALU.add)

            # ---- phase 2: attention per head-pair ----
            with ExitStack() as p2:
                sp = p2.enter_context(tc.tile_pool(name="spsum", bufs=2, space="PSUM"))
                vp = p2.enter_context(tc.tile_pool(name="vpsum", bufs=2, space="PSUM"))
                pp = p2.enter_context(tc.tile_pool(name="probs", bufs=6))
                npl = p2.enter_context(tc.tile_pool(name="norm", bufs=1))
                op = p2.enter_context(tc.tile_pool(name="oev", bufs=4))

                # Software-pipelined PV emission (see module docstring).
                pend = []

                def flush_pv():
                    while pend:
                        po_t, v_ap, pr_t, st, stp = pend.pop()
                        for i2 in range(IT):
                            nc.tensor.matmul(
                                po_t[:, i2 * 512:(i2 + 1) * 512],
                                v_ap,
                                pr_t[:, i2 * 512:(i2 + 1) * 512],
                                start=st, stop=stp)

                nm2 = None  # last pair's normalized upper half (base 0)
                for p in range(MT):  # head pair p: heads 2p, 2p+1
                    po = [vp.tile([65, S], F32, tag="pv", name="po") for _ in range(2)]
                    for j in range(JT):
                        for hh in range(2):
                            h = 2 * p + hh
                            sm = sp.tile([128, S], F32, tag="s")
                            for i2 in range(IT):
                                nc.tensor.matmul(
                                    sm[:, i2 * 512:(i2 + 1) * 512],
                                    KTt[p][hh * 64:(hh + 1) * 64, j * 128:(j + 1) * 128],
                                    QT[p][hh * 64:(hh + 1) * 64, i2 * 512:(i2 + 1) * 512],
                                    start=True, stop=False,
                                    tile_position=(hh * 64, 0))
                            # all three bias terms in one fp8 DoubleRow pass
                            for i2 in range(IT):
                                nc.tensor.matmul(
                                    sm[:, i2 * 512:(i2 + 1) * 512],
                                    ltv[:, h, j],
                                    drv[:, j, :, i2 * 512:(i2 + 1) * 512],
                                    start=False, stop=True,
                                    perf_mode=DR_MODE)
                            flush_pv()
                            pr = pp.tile([128, S], BF16, tag="pr")
                            nc.scalar.activation(
                                pr[:], sm[:], AF.Exp, bias=c0[:, h:h + 1],
                                scale=0.125)
                            pend.append((po[hh], V[j][:, h * 65:(h + 1) * 65],
                                         pr, j == 0, j == JT - 1))
                    flush_pv()
                    # Evict PV psums to SBUF so the next pair's (or phase
                    # 3's) psum ring frees up, then normalize off the SBUF
                    # copies.
                    last = (p == MT - 1)
                    xo = [npl.tile([65, S], F32, tag=f"xo{hh}", name="xo",
                                   bufs=2) for hh in range(2)]
                    if last:
                        # parallel eviction (ACT is done with exps by now)
                        nc.vector.tensor_copy(xo[0][:], po[0][:])
                        nc.scalar.copy(xo[1][:], po[1][:])
                    else:
                        for hh in range(2):
                            nc.vector.tensor_copy(xo[hh][:], po[hh][:])
                    dn = [xo[hh][64:65, :] for hh in range(2)]
                    # A [1, S] reciprocal is single-lane-serial on DVE
                    # (~6.5us); bounce both heads' denominator rows through a
                    # [128, 16] layout so all lanes work (~0.2us).
                    rt = npl.tile([128, 16], F32, tag="rt", bufs=2)
                    for hh in range(2):
                        nc.sync.dma_start(rt[:, hh * 8:(hh + 1) * 8],
                                          dn[hh][:])
                    rr = npl.tile([128, 16], BF16 if last else F32,
                                  tag="rr", bufs=2)
                    if last:
                        with nc.allow_low_precision(
                                reason="1/den only needs ~bf16 accuracy"):
                            nc.vector.reciprocal(rr[:], rt[:])
                    else:
                        nc.vector.reciprocal(rr[:], rt[:])
                    rc = npl.tile([1, 2 * S], BF16 if last else F32,
                                  tag="rc", bufs=2)
                    for hh in range(2):
                        nc.sync.dma_start(rc[:, hh * S:(hh + 1) * S],
                                          rr[:, hh * 8:(hh + 1) * 8])
                    if last:
                        # Keep the PE fed (and clocked up) through the
                        # normalization bounce: the first two output
                        # projection chains' xT0/xT1 matmuls only need the
                        # freed PV slots, and a burst of no-op filler
                        # matmuls bridges the remaining DMA latency.
                        p3chains = list(range(E // 128))
                        p3accs = []

                        def p3_open(e):
                            acc = vp.tile([128, S], F32, tag="pv", name="acc")
                            for m in range(MT - 1):
                                for i2 in range(IT):
                                    nc.tensor.matmul(
                                        acc[:, i2 * 512:(i2 + 1) * 512],
                                        wov[:, m, e * 128:(e + 1) * 128],
                                        xT[m][:, i2 * 512:(i2 + 1) * 512],
                                        start=(m == 0), stop=False)
                            return acc

                        def p3_close(acc, e):
                            # pair 2 contributes as two 64-deep matmuls: the
                            # lower half from xT[2][0:64], the upper half
                            # read straight from nm2 (partition base 0, no
                            # partition-shifting DMA needed).
                            for i2 in range(IT):
                                nc.tensor.matmul(
                                    acc[:, i2 * 512:(i2 + 1) * 512],
                                    wov[0:64, MT - 1, e * 128:(e + 1) * 128],
                                    xT[MT - 1][0:64, i2 * 512:(i2 + 1) * 512],
                                    start=False, stop=False)
                            for i2 in range(IT):
                                nc.tensor.matmul(
                                    acc[:, i2 * 512:(i2 + 1) * 512],
                                    woT2b[:, e * 128:(e + 1) * 128],
                                    nm2[:, i2 * 512:(i2 + 1) * 512],
                                    start=False, stop=True)
                            ev = op.tile([128, S], BF16, tag="ev")
                            # halves on both engines so the drain overlaps
                            nc.scalar.copy(ev[:, 0:512], acc[:, 0:512])
                            nc.vector.tensor_copy(ev[:, 512:1024],
                                                  acc[:, 512:1024])
                            nc.sync.dma_start(
                                y_out[e * 128:(e + 1) * 128, :], ev[:])

                        for _ in range(2):
                            e = p3chains.pop(0)
                            p3accs.append((p3_open(e), e))

                        # filler: writes a dead psum slot, no data deps
                        fill = sp.tile([64, S], F32, tag="s", name="fill")
                        for _ in range(20):
                            nc.tensor.matmul(fill[:, 0:512], ones64[:],
                                             QT[0][0:1, 0:512],
                                             start=True, stop=True)

                        # Broadcast 1/den to 64 partitions with a ones-row
                        # matmul per head into the freed score psum slots —
                        # beats the ~3.3us gpsimd partition_broadcast on the
                        # critical tail.
                        rbp = [sp.tile([64, S], F32, tag="s", name="rbp")
                               for _ in range(2)]
                        for hh in range(2):
                            for i2 in range(IT):
                                nc.tensor.matmul(
                                    rbp[hh][:, i2 * 512:(i2 + 1) * 512],
                                    ones64[:],
                                    rc[:, hh * S + i2 * 512:
                                       hh * S + (i2 + 1) * 512],
                                    start=True, stop=True)
                        # hh=0 first: phase 3's m2-lower matmuls depend on it
                        nc.vector.tensor_tensor(
                            xT[p][0:64, :], xo[0][0:64, :], rbp[0][:],
                            ALU.mult)
                        nm2 = npl.tile([64, S], BF16, tag="nm", bufs=2)
                        nc.vector.tensor_tensor(
                            nm2[:], xo[1][0:64, :], rbp[1][:], ALU.mult)

                        # ---- phase 3: output projection (inside the p2
                        # pools: accs ride the PV psum ring, no barrier) ----
                        while p3accs or p3chains:
                            if p3accs:
                                acc, e = p3accs.pop(0)
                                p3_close(acc, e)
                            if p3chains:
                                e = p3chains.pop(0)
                                p3accs.append((p3_open(e), e))
                    else:
                        rb = npl.tile([64, 2 * S], F32, tag="rb", bufs=2)
                        nc.gpsimd.partition_broadcast(rb[:], rc[:])
                        nc.vector.tensor_tensor(
                            xT[p][0:64, :], xo[0][0:64, :], rb[:, 0:S],
                            ALU.mult)
                        # DVE cannot shift partitions (and GpSimd cannot read
                        # PSUM): normalize at base 0 then DMA into 64-127.
                        nm = npl.tile([64, S], BF16, tag="nm", bufs=2)
                        nc.vector.tensor_tensor(
                            nm[:], xo[1][0:64, :], rb[:, S:2 * S], ALU.mult)
                        nc.sync.dma_start(xT[p][64:128, :], nm[:])

    nc.compile()
    _CACHE["nc"] = nc
    return nc


def _prep_core_inputs(hs_b, pos_row_b, pos_col_b, q_w, q_b, k_w, k_b, v_w,
                      rel_table, o_w, g):
    gsl = slice(g * GD, (g + 1) * GD)
    bf = ml_dtypes.bfloat16
    f8 = ml_dtypes.float8_e4m3
    hsT = np.ascontiguousarray(hs_b.T).astype(bf)
    wqT = np.ascontiguousarray(q_w[gsl, :].T).astype(bf)
    wkT = np.ascontiguousarray(k_w[gsl, :].T).astype(bf)
    wvT = np.ascontiguousarray(v_w[gsl, :].T).astype(bf)
    woT = np.ascontiguousarray(o_w[:, gsl].T).astype(bf)
    pr = np.asarray(pos_row_b).astype(np.int64)
    pc = np.asarray(pos_col_b).astype(np.int64)
    rpt = np.zeros((RP, S), np.float32)
    rpt[pr, np.arange(S)] = 1.0
    rpt[NROW + pc, np.arange(S)] = 1.0
    rptg = np.zeros((64, 2, S), np.float32)
    rptg[:, 0, :] = rpt[0:NROW, :]
    rptg[0:NCOL, 1, :] = rpt[NROW:RP, :]
    rpt96 = np.zeros((128, S), np.float32)
    rpt96[0:RP, :] = rpt
    rpt96r = np.broadcast_to(rpt96[:, None, :], (128, JT, S)).reshape(128, JT * S)
    meta = np.zeros((128, 2 * MT + HPG), np.float32)
    meta[:, 0:MT] = q_b[gsl].astype(np.float32).reshape(MT, 128).T
    meta[:, MT:2 * MT] = k_b[gsl].astype(np.float32).reshape(MT, 128).T
    t = rel_table[:, g * HPG:(g + 1) * HPG].astype(np.float32)  # [4, 6]
    c0v, c1v = t[0], t[1] - t[0]
    c2v, c3v = t[2] - t[0], t[3] - t[1] - t[2] + t[0]
    meta[:, 2 * MT:] = c0v[None, :]
    # DoubleRow lhsT pairs, coefficients x8 (exp applies scale=1/8)
    lt = np.zeros((128, HPG, JT, 2, 128), np.float32)
    eye = np.eye(128, dtype=np.float32)
    rps = np.zeros((HPG, 128, S), np.float32)
    rps[:, 0:NROW, :] = 8.0 * c1v[:, None, None] * rpt[None, 0:NROW, :]
    rps[:, NROW:RP, :] = 8.0 * c2v[:, None, None] * rpt[None, NROW:RP, :]
    for h in range(HPG):
        for j in range(JT):
            lt[:, h, j, 0, :] = 8.0 * c3v[h] * eye
            lt[:, h, j, 1, :] = rps[h][:, j * 128:(j + 1) * 128]
    ones64 = np.ones((1, 64), ml_dtypes.bfloat16)

    return {
        "hsT": hsT, "wqT": wqT, "wkT": wkT, "wvT": wvT, "woT": woT,
        "rptg": np.ascontiguousarray(rptg.reshape(64, -1)).astype(f8),
        "rpt96r": np.ascontiguousarray(rpt96r).astype(f8),
        "lt": np.ascontiguousarray(lt.reshape(128, -1)).astype(f8),
        "meta": meta, "ones64": ones64,
    }


def make_in_maps(hidden_states, pos_row, pos_col, q_w, q_b, k_w, k_b, v_w,
                 rel_table, o_w):
    in_maps = []
    for c in range(8):
        b, g = c // HG, c % HG
        in_maps.append(_prep_core_inputs(
            hidden_states[b], pos_row[b], pos_col[b], q_w, q_b, k_w, k_b,
            v_w, rel_table, o_w, g))
    return in_maps


def assemble(results, v_b, o_w, o_b):
    # v_b contributes exactly v_b @ o_w_g.T per group (softmax rows sum to 1)
    bias_row = o_b.copy()
    for g in range(HG):
        gsl = slice(g * GD, (g + 1) * GD)
        bias_row = bias_row + v_b[gsl] @ o_w[:, gsl].T
    out = np.empty((B, S, E), np.float32)
    for b in range(B):
        out[b] = (np.asarray(results[2 * b]["outp"], np.float32).T
                  + np.asarray(results[2 * b + 1]["outp"], np.float32).T
                  + bias_row[None, :])
    return out


def kernel(hidden_states, pos_row, pos_col, q_w, q_b, k_w, k_b, v_w, v_b,
           o_w, o_b, rel_table):
    hidden_states = np.asarray(hidden_states, dtype=np.float32)
    q_w = np.asarray(q_w, dtype=np.float32); q_b = np.asarray(q_b, dtype=np.float32)
    k_w = np.asarray(k_w, dtype=np.float32); k_b = np.asarray(k_b, dtype=np.float32)
    v_w = np.asarray(v_w, dtype=np.float32); v_b = np.asarray(v_b, dtype=np.float32)
    o_w = np.asarray(o_w, dtype=np.float32); o_b = np.asarray(o_b, dtype=np.float32)
    rel_table = np.asarray(rel_table, dtype=np.float32)

    nc = build_nc()
    in_maps = make_in_maps(hidden_states, pos_row, pos_col, q_w, q_b, k_w,
                           k_b, v_w, rel_table, o_w)
    res = run_bass_kernel_spmd(nc, in_maps, core_ids=list(range(8)))
    return assemble(res.results, v_b, o_w, o_b)
